# revision 1
# baseline (speedup 1.0000x reference)
"""Trainium2 Bass kernel for nn_Composer (gnn_message_passing).

Math (exact reformulation of the reference):
  out[b,s1,:] = (heads[b,s1]==0) * ( base + sum_{s2: heads[b,s2]==s1} w[s2]*(t_on[b,s2]-t_off) )
  t_on[b,s2]  = tanh(u[b,s2] + bc),  u[b,s2,o] = tok[b,s2] @ Wc[o] @ tanh(tok[b,s2])
  t_off       = tanh(bc),  base = t_off*sum(w) + br

Only rows s2 whose head lands on a row with head==0 contribute to the output,
so u is needed for a handful of rows (R ~ 4-16 of 4096). The unavoidable cost
is streaming the 226 MB bilinear weight Wc once. Sharding: Wc is split over
the output dim O=384 across 8 cores (48 each, 28.3 MB/core); every core
computes its o-slice of u for all selected rows via 3 accumulated matmuls per
output channel (contraction d on partitions, Wc streamed as the moving
operand), then a fused multiply+reduce against dep on the vector engine.
The host does index selection, sharding, and the final scatter of the ~R
result vectors into the zero output.
"""
import numpy as np

import concourse.bass as bass
import concourse.bacc as bacc
import concourse.mybir as mybir
from concourse.tile import TileContext
from concourse.tile_rust import add_dep_helper
from concourse.bass_utils import run_bass_kernel_spmd

F32 = mybir.dt.float32
F32R = mybir.dt.float32r

B, S, D = 8, 512, 384
NCORES = 8
OC = D // NCORES          # output channels per core = 48
DC = D // 128             # contraction chunks = 3
R_MAX = 64                # padded selected-row capacity per device run
# Wc transfer group sizes (in output channels): small head groups so compute
# starts early, big middle groups for DMA efficiency, small tail groups so the
# final DMA->compute->epilogue chain is short.
GROUP_SIZES = [1, 2] + [3] * 14 + [2, 1]
assert sum(GROUP_SIZES) == OC
N_GRP = len(GROUP_SIZES)
WC_BUFS = 6

_nc_cache = {}


def _build_nc():
    if "nc" in _nc_cache:
        return _nc_cache["nc"]
    nc = bacc.Bacc("TRN2", target_bir_lowering=False, debug=False)
    wc_d = nc.dram_tensor("wc", [OC, 128, DC * 384], F32R,
                          kind="ExternalInput")
    tokT_d = nc.dram_tensor("tokT", [128, DC * R_MAX], F32R, kind="ExternalInput")
    tok_d = nc.dram_tensor("tok", [R_MAX, D], F32, kind="ExternalInput")
    w_d = nc.dram_tensor("w", [R_MAX, 1], F32, kind="ExternalInput")
    bcr_d = nc.dram_tensor("bcrep", [128, OC], F32, kind="ExternalInput")
    contrib_d = nc.dram_tensor("contrib", [R_MAX, OC], F32, kind="ExternalOutput")
    toff_d = nc.dram_tensor("toff", [1, OC], F32, kind="ExternalOutput")

    AF = mybir.ActivationFunctionType
    OP = mybir.AluOpType

    HOC = OC // 2             # 24 output channels per epilogue half

    with TileContext(nc) as tc:
        with (
            tc.tile_pool(name="const", bufs=1) as cp,
            tc.tile_pool(name="wcp", bufs=WC_BUFS) as wcp,
            tc.tile_pool(name="zp", bufs=16) as zp,
            tc.tile_pool(name="pp", bufs=4, space="PSUM") as pp,
        ):
            offs = [sum(GROUP_SIZES[:g]) for g in range(N_GRP)]

            def wc_dma(g, wt):
                no = GROUP_SIZES[g]
                nc.sync.dma_start(
                    out=wt[:].rearrange("p (o f) -> p o f", o=no),
                    in_=wc_d[offs[g]:offs[g] + no].rearrange("o p f -> p o f"))

            # Wc stream owns the SP HWDGE ring; everything small goes through
            # the scalar engine's ring so it never queues behind megabytes.
            wts = []
            for g in range(N_GRP):
                wts.append(wcp.tile([128, GROUP_SIZES[g] * DC * 384], F32R,
                                    tag="wc", name=f"wt{g}"))
            for g in range(WC_BUFS):
                wc_dma(g, wts[g])

            tokT_sb = cp.tile([128, DC * R_MAX], F32R)
            nc.scalar.dma_start(out=tokT_sb[:], in_=tokT_d[:])
            tok_sb = cp.tile([R_MAX, D], F32)
            nc.scalar.dma_start(out=tok_sb[:], in_=tok_d[:])
            w_sb = cp.tile([R_MAX, 1], F32)
            nc.scalar.dma_start(out=w_sb[:], in_=w_d[:])
            bcr_sb = cp.tile([128, OC], F32)
            nc.scalar.dma_start(out=bcr_sb[:], in_=bcr_d[:])

            dep_sb = cp.tile([R_MAX, D], F32)
            nc.scalar.activation(dep_sb[:], tok_sb[:], AF.Tanh)
            toff_sb = cp.tile([128, OC], F32)
            nc.scalar.activation(toff_sb[:], bcr_sb[:], AF.Tanh)
            nc.scalar.dma_start(out=toff_d[:], in_=toff_sb[0:1, :])
            # DVE observes dep/w/bcr ticks here so the hot-loop reduce ops
            # carry few sync waits (each extra wait costs an event semaphore)
            dep_touch = cp.tile([R_MAX, 1], F32)
            nc.vector.tensor_copy(out=dep_touch[:], in_=dep_sb[:, 0:1])
            # toffw[r,o] = tanh(bc)[o] * w[r], independent of u — compute early
            toffw_sb = cp.tile([R_MAX, OC], F32)
            nc.vector.tensor_scalar_mul(toffw_sb[:], toff_sb[0:R_MAX, :], w_sb[:])

            u_half = [cp.tile([R_MAX, HOC], F32, tag="u0", name="u0"),
                      cp.tile([R_MAX, HOC], F32, tag="u1", name="u1")]

            def epilogue(lo, hi):
                """contrib[:, lo:hi] = w*(tanh(u+bc) - t_off). For a single
                channel the +bc folds into the ACT bias port (bc is constant
                across partitions), skipping the DVE add."""
                n = hi - lo
                ton = cp.tile([R_MAX, n], F32, tag=f"ton{lo}", name=f"ton{lo}")
                uv = (u_half[0][:, lo:hi] if hi <= HOC
                      else u_half[1][:, lo - HOC:hi - HOC])
                if n == 1:
                    nc.scalar.activation(ton[:], uv, AF.Tanh,
                                         bias=bcr_sb[0:R_MAX, lo:lo + 1])
                else:
                    nc.vector.tensor_tensor(ton[:], uv,
                                            bcr_sb[0:R_MAX, lo:hi], OP.add)
                    nc.scalar.activation(ton[:], ton[:], AF.Tanh)
                csb = cp.tile([R_MAX, n], F32, tag=f"c{lo}", name=f"c{lo}")
                # contrib = t_on*w - t_off*w
                nc.vector.scalar_tensor_tensor(
                    out=csb[:], in0=ton[:], scalar=w_sb[:],
                    in1=toffw_sb[:, lo:hi],
                    op0=OP.mult, op1=OP.subtract)
                nc.scalar.dma_start(out=contrib_d[:, lo:hi], in_=csb[:])

            for g in range(N_GRP):
                if g >= WC_BUFS:
                    wc_dma(g, wts[g])
                wt = wts[g]
                for oi in range(GROUP_SIZES[g]):
                    o = offs[g] + oi
                    ps = pp.tile([R_MAX, 384], F32, tag="ps")
                    for c in range(DC):
                        nc.tensor.matmul(
                            ps[:],
                            lhsT=tokT_sb[:, c * R_MAX:(c + 1) * R_MAX],
                            rhs=wt[:, (oi * DC + c) * 384:(oi * DC + c + 1) * 384],
                            start=(c == 0), stop=(c == DC - 1),
                        )
                    z = zp.tile([R_MAX, 384], F32, tag="z")
                    nc.vector.scalar_tensor_tensor(
                        out=z[:], in0=ps[:], scalar=1.0, in1=dep_sb[:],
                        op0=OP.mult, op1=OP.mult,
                        accum_out=u_half[o // HOC][:, o % HOC:o % HOC + 1],
                    )
                    if o == HOC - 1:
                        epilogue(0, HOC)
                    elif o == OC - 2:
                        epilogue(HOC, OC - 1)
            epilogue(OC - 1, OC)

    nc.compile()
    _nc_cache["nc"] = nc
    return nc


def _shard_wc(Wc):
    """Per-core Wc layout: [OC, 128(p), DC*384] with d = c*128 + p,
    free index = c*384 + e."""
    shards = []
    for k in range(NCORES):
        wck = Wc[k * OC:(k + 1) * OC]                       # [48, 384, 384]
        wck = wck.reshape(OC, DC, 128, 384)
        wck = np.ascontiguousarray(wck.transpose(0, 2, 1, 3))
        shards.append(wck.reshape(OC, 128, DC * 384))
    return shards


def run_device(in_maps, trace=False, tmpdir=None):
    nc = _build_nc()
    return run_bass_kernel_spmd(nc, in_maps, list(range(NCORES)),
                                trace=trace, tmpdir=tmpdir)


def _make_in_maps(tok_sel, w_sel, wc_shards, bc):
    """tok_sel [R_MAX, D] f32, w_sel [R_MAX] f32."""
    # tokT[p, c*R_MAX + r] = tok_sel[r, c*128 + p]
    tokT = np.ascontiguousarray(
        tok_sel.T.reshape(DC, 128, R_MAX).transpose(1, 0, 2)
    ).reshape(128, DC * R_MAX)
    maps = []
    for k in range(NCORES):
        maps.append({
            "wc": wc_shards[k],
            "tokT": tokT,
            "tok": tok_sel,
            "w": w_sel.reshape(R_MAX, 1),
            "bcrep": np.ascontiguousarray(
                np.broadcast_to(bc[k * OC:(k + 1) * OC], (128, OC))),
        })
    return maps


def kernel(**inputs):
    tokens = np.asarray(inputs["tokens"])
    heads = np.asarray(inputs["dep_heads"])
    tok_table = np.asarray(inputs["tok_table"], dtype=np.float32)
    Wc = np.asarray(inputs["Wc"], dtype=np.float32)
    bc = np.asarray(inputs["bc"], dtype=np.float32)
    Wr = np.asarray(inputs["Wr"], dtype=np.float32)
    br = np.asarray(inputs["br"], dtype=np.float32)
    assert tokens.shape == (B, S) and Wc.shape == (D, D, D)

    # host index selection: rows that can reach an unmasked (head==0) output row
    zs = [np.nonzero(heads[b] == 0)[0] for b in range(B)]
    sel = [(b, int(s2), int(heads[b, s2]))
           for b in range(B)
           for s2 in np.nonzero(np.isin(heads[b], zs[b]))[0]]
    R = len(sel)

    wc_shards = _shard_wc(Wc)
    w_full = Wr[0]

    contribs = []
    toff = None
    for lo in range(0, max(R, 1), R_MAX):
        chunk = sel[lo:lo + R_MAX]
        tok_sel = np.zeros((R_MAX, D), dtype=np.float32)
        w_sel = np.zeros(R_MAX, dtype=np.float32)
        for i, (b, s2, _dest) in enumerate(chunk):
            tok_sel[i] = tok_table[tokens[b, s2]]
            w_sel[i] = w_full[s2]
        res = run_device(_make_in_maps(tok_sel, w_sel, wc_shards, bc)).results
        contribs.append(np.concatenate(
            [res[k]["contrib"] for k in range(NCORES)], axis=1))
        toff = np.concatenate([res[k]["toff"][0] for k in range(NCORES)])

    base = (toff * w_full.sum() + br[0]).astype(np.float32)
    out = np.zeros((B, S, D), dtype=np.float32)
    for b in range(B):
        out[b, zs[b]] = base
    for i, (b, _s2, dest) in enumerate(sel):
        out[b, dest] += contribs[i // R_MAX][i % R_MAX]
    return out



# revision 2
# speedup vs baseline: 2.2682x; 2.2682x over previous
"""Trainium2 Bass kernel for nn_Composer (gnn_message_passing).

Math (exact reformulation of the reference):
  out[b,s1,:] = (heads[b,s1]==0) * ( base + sum_{s2: heads[b,s2]==s1} w[s2]*(t_on[b,s2]-t_off) )
  t_on[b,s2]  = tanh(u[b,s2] + bc),  u[b,s2,o] = tok[b,s2] @ Wc[o] @ tanh(tok[b,s2])
  t_off       = tanh(bc),  base = t_off*sum(w) + br

Only rows s2 whose head lands on a row with head==0 contribute to the output,
so u is needed for a handful of rows (R ~ 4-16 of 4096). The unavoidable cost
is streaming the bilinear weight Wc once; it is quantized to fp8e4 on the host
(226 MB f32 -> 56.6 MB fp8; the bilinear term is a small correction on top of
the exactly-computed base, so e4m3 error lands ~1e-3 of the output scale, far
under the 2e-2 gate). Wc is scaled by 8 before quantization to keep values out
of the fp8 subnormal range; the 1/8 is folded into dep on the host.

Sharding: Wc split over the output dim O=384 across 8 cores (48 each, 7.08 MB
fp8/core). Each core computes its o-slice of u for all selected rows with
column-tiled matmul pairs: o-channel 2j runs in PE array column groups 0-1
(PSUM partitions 0-63) while 2j+1 runs concurrently in groups 2-3 (partitions
64-127), both streaming their fp8 Wc slice as the moving operand against the
same bf16 tokT stationary chunk. A fused DVE multiply+reduce against dep
(stacked twice across partitions) produces u for both channels at once.
The host does index selection, quantization, sharding, and the final scatter.
"""
import numpy as np
import ml_dtypes

import concourse.bass as bass
import concourse.bacc as bacc
import concourse.mybir as mybir
from concourse.tile import TileContext
from concourse.tile_rust import add_dep_helper
from concourse.bass_utils import run_bass_kernel_spmd

F32 = mybir.dt.float32
BF16 = mybir.dt.bfloat16
FP8 = mybir.dt.float8e4

B, S, D = 8, 512, 384
NCORES = 8
OC = D // NCORES          # output channels per core = 48
NP = OC // 2              # column-tiled o-channel pairs per core = 24
DC = D // 128             # contraction chunks = 3
FR = DC * 384             # fp8 free-dim elements per o-channel = 1152
R_MAX = 64                # padded selected-row capacity per device run
SCALE = 8.0               # host folds Wc*8 / dep/8 to avoid fp8 subnormals
# Wc transfer group sizes in o-channels (even: channels are consumed in
# column-tiled pairs). Small head group so compute starts early, big middle
# groups for DMA efficiency, small tail so the last DMA->compute chain is
# short.
GROUP_SIZES = [2, 4, 6, 8, 8, 8, 6, 4, 2]
assert sum(GROUP_SIZES) == OC and all(g % 2 == 0 for g in GROUP_SIZES)
N_GRP = len(GROUP_SIZES)
WC_BUFS = 4

_nc_cache = {}


def _build_nc():
    if "nc" in _nc_cache:
        return _nc_cache["nc"]
    nc = bacc.Bacc("TRN2", target_bir_lowering=False, debug=False)
    wc_d = nc.dram_tensor("wc", [128, OC * FR], FP8, kind="ExternalInput")
    tokT_d = nc.dram_tensor("tokT", [128, DC * R_MAX], BF16, kind="ExternalInput")
    dep2_d = nc.dram_tensor("dep2", [128, D], BF16, kind="ExternalInput")
    w2_d = nc.dram_tensor("w2", [128, 1], F32, kind="ExternalInput")
    bcr2_d = nc.dram_tensor("bcr2", [128, NP], F32, kind="ExternalInput")
    contrib_d = nc.dram_tensor("contrib", [128, NP], F32, kind="ExternalOutput")
    toff_d = nc.dram_tensor("toff", [2, NP], F32, kind="ExternalOutput")

    AF = mybir.ActivationFunctionType
    OP = mybir.AluOpType

    # epilogue chunk boundaries in pair index space: [0,12), [12,23), [23,24)
    EP = [(0, 12), (12, 23), (23, 24)]

    with TileContext(nc) as tc:
        with (
            tc.tile_pool(name="const", bufs=1) as cp,
            tc.tile_pool(name="wcp", bufs=WC_BUFS) as wcp,
            tc.tile_pool(name="zp", bufs=16) as zp,
            tc.tile_pool(name="pp", bufs=4, space="PSUM") as pp,
        ):
            offs = [sum(GROUP_SIZES[:g]) for g in range(N_GRP)]

            def wc_dma(g, wt):
                no = GROUP_SIZES[g]
                nc.sync.dma_start(
                    out=wt[:],
                    in_=wc_d[:, offs[g] * FR:(offs[g] + no) * FR])

            # Wc stream owns the SP HWDGE ring; everything small goes through
            # the scalar engine's ring so it never queues behind megabytes.
            wts = []
            for g in range(N_GRP):
                wts.append(wcp.tile([128, GROUP_SIZES[g] * FR], FP8,
                                    tag="wc", name=f"wt{g}"))
            for g in range(WC_BUFS):
                wc_dma(g, wts[g])

            tokT_sb = cp.tile([128, DC * R_MAX], BF16)
            nc.scalar.dma_start(out=tokT_sb[:], in_=tokT_d[:])
            dep2_sb = cp.tile([128, D], BF16)
            nc.scalar.dma_start(out=dep2_sb[:], in_=dep2_d[:])
            w2_sb = cp.tile([128, 1], F32)
            nc.scalar.dma_start(out=w2_sb[:], in_=w2_d[:])
            bcr2_sb = cp.tile([128, NP], F32)
            nc.scalar.dma_start(out=bcr2_sb[:], in_=bcr2_d[:])

            toff2_sb = cp.tile([128, NP], F32)
            nc.scalar.activation(toff2_sb[:], bcr2_sb[:], AF.Tanh)
            nc.scalar.dma_start(out=toff_d[:], in_=toff2_sb[0:128:64, :])
            # DVE observes dep2/w2 ticks here so the hot-loop reduce ops
            # carry few sync waits (each extra wait costs an event semaphore)
            dep_touch = cp.tile([128, 1], F32)
            nc.vector.tensor_copy(out=dep_touch[:], in_=dep2_sb[:, 0:1])
            # toffw2[p,j] = tanh(bc)[pair j, half p//64] * w[p%64]
            toffw2_sb = cp.tile([128, NP], F32)
            nc.vector.tensor_scalar_mul(toffw2_sb[:], toff2_sb[:], w2_sb[:])

            u_t = [cp.tile([128, hi - lo], F32, tag=f"u{lo}", name=f"u{lo}")
                   for lo, hi in EP]

            def epilogue(ei):
                """contrib[:, lo:hi] = w*(tanh(u+bc) - t_off). For a single
                pair the +bc folds into the ACT bias port."""
                lo, hi = EP[ei]
                n = hi - lo
                ton = cp.tile([128, n], F32, tag=f"ton{lo}", name=f"ton{lo}")
                if n == 1:
                    nc.scalar.activation(ton[:], u_t[ei][:], AF.Tanh,
                                         bias=bcr2_sb[:, lo:lo + 1])
                else:
                    nc.vector.tensor_tensor(ton[:], u_t[ei][:],
                                            bcr2_sb[:, lo:hi], OP.add)
                    nc.scalar.activation(ton[:], ton[:], AF.Tanh)
                csb = cp.tile([128, n], F32, tag=f"c{lo}", name=f"c{lo}")
                # contrib = t_on*w - t_off*w
                nc.vector.scalar_tensor_tensor(
                    out=csb[:], in0=ton[:], scalar=w2_sb[:],
                    in1=toffw2_sb[:, lo:hi],
                    op0=OP.mult, op1=OP.subtract)
                nc.scalar.dma_start(out=contrib_d[:, lo:hi], in_=csb[:])

            ep_next = 0
            for g in range(N_GRP):
                if g >= WC_BUFS:
                    wc_dma(g, wts[g])
                wt = wts[g]
                for pi in range(GROUP_SIZES[g] // 2):
                    j = offs[g] // 2 + pi          # global pair index
                    oa, ob = 2 * pi, 2 * pi + 1    # local o within group
                    ps = pp.tile([128, 384], F32, tag="ps")
                    for c in range(DC):
                        nc.tensor.matmul(
                            ps[0:64, :],
                            lhsT=tokT_sb[:, c * R_MAX:(c + 1) * R_MAX],
                            rhs=wt[:, oa * FR + c * 384:oa * FR + (c + 1) * 384],
                            start=(c == 0), stop=(c == DC - 1),
                            tile_position=(0, 0),
                        )
                        nc.tensor.matmul(
                            ps[64:128, :],
                            lhsT=tokT_sb[:, c * R_MAX:(c + 1) * R_MAX],
                            rhs=wt[:, ob * FR + c * 384:ob * FR + (c + 1) * 384],
                            start=(c == 0), stop=(c == DC - 1),
                            tile_position=(0, 64),
                        )
                    z = zp.tile([128, 384], F32, tag="z")
                    ei = next(i for i, (lo, hi) in enumerate(EP) if j < hi)
                    lo = EP[ei][0]
                    nc.vector.scalar_tensor_tensor(
                        out=z[:], in0=ps[:], scalar=1.0, in1=dep2_sb[:],
                        op0=OP.mult, op1=OP.mult,
                        accum_out=u_t[ei][:, j - lo:j - lo + 1],
                    )
                    if ep_next < len(EP) and j == EP[ep_next][1] - 1:
                        epilogue(ep_next)
                        ep_next += 1

    nc.compile()
    _nc_cache["nc"] = nc
    return nc


def _shard_wc(Wc):
    """Per-core Wc layout: [128(p), OC*FR] fp8e4, scaled by 8.
    free index = o*1152 + c*384 + e with d = c*128 + p."""
    shards = []
    for k in range(NCORES):
        wck = (Wc[k * OC:(k + 1) * OC] * SCALE).astype(ml_dtypes.float8_e4m3)
        wck = wck.reshape(OC, DC, 128, 384).transpose(2, 0, 1, 3)
        shards.append(np.ascontiguousarray(wck).reshape(128, OC * FR))
    return shards


def run_device(in_maps, trace=False, tmpdir=None):
    nc = _build_nc()
    return run_bass_kernel_spmd(nc, in_maps, list(range(NCORES)),
                                trace=trace, tmpdir=tmpdir)


def _make_in_maps(tok_sel, w_sel, wc_shards, bc):
    """tok_sel [R_MAX, D] f32, w_sel [R_MAX] f32."""
    # tokT[p, c*R_MAX + r] = tok_sel[r, c*128 + p]
    tokT = np.ascontiguousarray(
        tok_sel.T.reshape(DC, 128, R_MAX).transpose(1, 0, 2)
    ).reshape(128, DC * R_MAX).astype(ml_dtypes.bfloat16)
    dep = (np.tanh(tok_sel) / SCALE).astype(ml_dtypes.bfloat16)
    dep2 = np.concatenate([dep, dep], axis=0)              # [128, D]
    w2 = np.concatenate([w_sel, w_sel]).reshape(128, 1).astype(np.float32)
    maps = []
    for k in range(NCORES):
        bck = bc[k * OC:(k + 1) * OC]
        bcr2 = np.concatenate([
            np.broadcast_to(bck[0::2], (64, NP)),
            np.broadcast_to(bck[1::2], (64, NP))]).astype(np.float32)
        maps.append({
            "wc": wc_shards[k],
            "tokT": tokT,
            "dep2": dep2,
            "w2": w2,
            "bcr2": np.ascontiguousarray(bcr2),
        })
    return maps


def kernel(**inputs):
    tokens = np.asarray(inputs["tokens"])
    heads = np.asarray(inputs["dep_heads"])
    tok_table = np.asarray(inputs["tok_table"], dtype=np.float32)
    Wc = np.asarray(inputs["Wc"], dtype=np.float32)
    bc = np.asarray(inputs["bc"], dtype=np.float32)
    Wr = np.asarray(inputs["Wr"], dtype=np.float32)
    br = np.asarray(inputs["br"], dtype=np.float32)
    assert tokens.shape == (B, S) and Wc.shape == (D, D, D)

    # host index selection: rows that can reach an unmasked (head==0) output row
    zs = [np.nonzero(heads[b] == 0)[0] for b in range(B)]
    sel = [(b, int(s2), int(heads[b, s2]))
           for b in range(B)
           for s2 in np.nonzero(np.isin(heads[b], zs[b]))[0]]
    R = len(sel)

    wc_shards = _shard_wc(Wc)
    w_full = Wr[0]

    contribs = []
    toff = None
    for lo in range(0, max(R, 1), R_MAX):
        chunk = sel[lo:lo + R_MAX]
        tok_sel = np.zeros((R_MAX, D), dtype=np.float32)
        w_sel = np.zeros(R_MAX, dtype=np.float32)
        for i, (b, s2, _dest) in enumerate(chunk):
            tok_sel[i] = tok_table[tokens[b, s2]]
            w_sel[i] = w_full[s2]
        res = run_device(_make_in_maps(tok_sel, w_sel, wc_shards, bc)).results
        # contrib[p, j]: row r=p%64, local channel o=2j+(p//64)
        ck = []
        for k in range(NCORES):
            c2 = res[k]["contrib"]
            c = np.empty((R_MAX, OC), dtype=np.float32)
            c[:, 0::2] = c2[0:64]
            c[:, 1::2] = c2[64:128]
            ck.append(c)
        contribs.append(np.concatenate(ck, axis=1))        # [R_MAX, D]
        tk = []
        for k in range(NCORES):
            t2 = res[k]["toff"]                            # [2, NP]
            t = np.empty(OC, dtype=np.float32)
            t[0::2] = t2[0]
            t[1::2] = t2[1]
            tk.append(t)
        toff = np.concatenate(tk)                          # [D]

    base = (toff * w_full.sum() + br[0]).astype(np.float32)
    out = np.zeros((B, S, D), dtype=np.float32)
    for b in range(B):
        out[b, zs[b]] = base
    for i, (b, _s2, dest) in enumerate(sel):
        out[b, dest] += contribs[i // R_MAX][i % R_MAX]
    return out


# revision 3
# speedup vs baseline: 2.3816x; 1.0500x over previous
"""Trainium2 Bass kernel for nn_Composer (gnn_message_passing).

Math (exact reformulation of the reference):
  out[b,s1,:] = (heads[b,s1]==0) * ( base + sum_{s2: heads[b,s2]==s1} w[s2]*(t_on[b,s2]-t_off) )
  t_on[b,s2]  = tanh(u[b,s2] + bc),  u[b,s2,o] = tok[b,s2] @ Wc[o] @ tanh(tok[b,s2])
  t_off       = tanh(bc),  base = t_off*sum(w) + br

Only rows s2 whose head lands on a row with head==0 contribute to the output,
so u is needed for a handful of rows (R ~ 4-16 of 4096). The unavoidable cost
is streaming the bilinear weight Wc once; it is quantized to fp8e4 on the host
(226 MB f32 -> 56.6 MB fp8; the bilinear term is a small correction on top of
the exactly-computed base, so e4m3 error lands ~1e-3 of the output scale, far
under the 2e-2 gate). Wc is scaled by 8 before quantization to keep values out
of the fp8 subnormal range; the 1/8 is folded into dep on the host.

Sharding: Wc split over the output dim O=384 across 8 cores (48 each, 7.08 MB
fp8/core). Each core computes its o-slice of u with 4-way column-tiled
matmuls: o-channels 4j..4j+3 run concurrently in PE array column groups
0/1/2/3 (PSUM partition quarters), each streaming its fp8 Wc slice as the
moving operand against the same bf16 tokT stationary chunk. This keeps the
per-group tensor time under the DMA pace even when the HAM clock gate holds
the PE at 1.2 GHz (per-group DMA waits re-throttle it). A fused DVE
multiply+reduce against dep (stacked 4x across partitions) produces u for all
four channels at once. The host does index selection, quantization, sharding,
and the final scatter.
"""
import numpy as np
import ml_dtypes

import concourse.bass as bass
import concourse.bacc as bacc
import concourse.mybir as mybir
from concourse.tile import TileContext
from concourse.tile_rust import add_dep_helper
from concourse.bass_utils import run_bass_kernel_spmd

F32 = mybir.dt.float32
BF16 = mybir.dt.bfloat16
FP8 = mybir.dt.float8e4

B, S, D = 8, 512, 384
NCORES = 8
OC = D // NCORES          # output channels per core = 48
COLS = 4                  # column-tiled concurrent o-channels per wave
NQ = OC // COLS           # o-channel quads per core = 12
DC = D // 128             # contraction chunks = 3
FR = DC * 384             # fp8 free-dim elements per o-channel = 1152
R_MAX = 128 // COLS       # padded selected-row capacity per device run = 32
SCALE = 8.0               # host folds Wc*8 / dep/8 to avoid fp8 subnormals
# Wc transfer group sizes in o-channels (multiples of COLS: channels are
# consumed in column-tiled quads). Small head group so compute starts early,
# big middle groups for DMA efficiency, small tail so the last DMA->compute
# chain is short.
GROUP_SIZES = [4, 4, 8, 8, 8, 8, 4, 4]
assert sum(GROUP_SIZES) == OC and all(g % COLS == 0 for g in GROUP_SIZES)
N_GRP = len(GROUP_SIZES)
WC_BUFS = 4

_nc_cache = {}


def _build_nc():
    if "nc" in _nc_cache:
        return _nc_cache["nc"]
    nc = bacc.Bacc("TRN2", target_bir_lowering=False, debug=False)
    wc_d = nc.dram_tensor("wc", [OC, 128, FR], FP8, kind="ExternalInput")
    tokT_d = nc.dram_tensor("tokT", [128, DC * R_MAX], BF16, kind="ExternalInput")
    dep4_d = nc.dram_tensor("dep4", [128, D], BF16, kind="ExternalInput")
    w4_d = nc.dram_tensor("w4", [128, 1], F32, kind="ExternalInput")
    bcr4_d = nc.dram_tensor("bcr4", [128, NQ], F32, kind="ExternalInput")
    contrib_d = nc.dram_tensor("contrib", [128, NQ], F32, kind="ExternalOutput")
    toff_d = nc.dram_tensor("toff", [COLS, NQ], F32, kind="ExternalOutput")

    AF = mybir.ActivationFunctionType
    OP = mybir.AluOpType

    # epilogue chunk boundaries in quad index space
    EP = [(0, 6), (6, 11), (11, 12)]

    with TileContext(nc) as tc:
        with (
            tc.tile_pool(name="const", bufs=1) as cp,
            tc.tile_pool(name="wcp", bufs=WC_BUFS) as wcp,
            tc.tile_pool(name="zp", bufs=16) as zp,
            tc.tile_pool(name="pp", bufs=4, space="PSUM") as pp,
        ):
            offs = [sum(GROUP_SIZES[:g]) for g in range(N_GRP)]

            def wc_dma(g, wt):
                no = GROUP_SIZES[g]
                nc.sync.dma_start(
                    out=wt[:].rearrange("p (o f) -> p o f", o=no),
                    in_=wc_d[offs[g]:offs[g] + no].rearrange("o p f -> p o f"))

            # Wc stream owns the SP HWDGE ring; everything small goes through
            # the scalar engine's ring so it never queues behind megabytes.
            wts = []
            for g in range(N_GRP):
                wts.append(wcp.tile([128, GROUP_SIZES[g] * FR], FP8,
                                    tag="wc", name=f"wt{g}"))
            for g in range(WC_BUFS):
                wc_dma(g, wts[g])

            tokT_sb = cp.tile([128, DC * R_MAX], BF16)
            nc.scalar.dma_start(out=tokT_sb[:], in_=tokT_d[:])
            dep4_sb = cp.tile([128, D], BF16)
            nc.scalar.dma_start(out=dep4_sb[:], in_=dep4_d[:])
            w4_sb = cp.tile([128, 1], F32)
            nc.scalar.dma_start(out=w4_sb[:], in_=w4_d[:])
            bcr4_sb = cp.tile([128, NQ], F32)
            nc.scalar.dma_start(out=bcr4_sb[:], in_=bcr4_d[:])

            toff4_sb = cp.tile([128, NQ], F32)
            nc.scalar.activation(toff4_sb[:], bcr4_sb[:], AF.Tanh)
            nc.scalar.dma_start(out=toff_d[:], in_=toff4_sb[0:128:R_MAX, :])
            # DVE observes dep4/w4 ticks here so the hot-loop reduce ops
            # carry few sync waits (each extra wait costs an event semaphore)
            dep_touch = cp.tile([128, 1], F32)
            nc.vector.tensor_copy(out=dep_touch[:], in_=dep4_sb[:, 0:1])
            # toffw4[p,j] = tanh(bc)[quad j, col p//32] * w[p%32]
            toffw4_sb = cp.tile([128, NQ], F32)
            nc.vector.tensor_scalar_mul(toffw4_sb[:], toff4_sb[:], w4_sb[:])

            u_t = [cp.tile([128, hi - lo], F32, tag=f"u{lo}", name=f"u{lo}")
                   for lo, hi in EP]

            def epilogue(ei):
                """contrib[:, lo:hi] = w*(tanh(u+bc) - t_off). For a single
                quad the +bc folds into the ACT bias port."""
                lo, hi = EP[ei]
                n = hi - lo
                ton = cp.tile([128, n], F32, tag=f"ton{lo}", name=f"ton{lo}")
                if n == 1:
                    nc.scalar.activation(ton[:], u_t[ei][:], AF.Tanh,
                                         bias=bcr4_sb[:, lo:lo + 1])
                else:
                    nc.vector.tensor_tensor(ton[:], u_t[ei][:],
                                            bcr4_sb[:, lo:hi], OP.add)
                    nc.scalar.activation(ton[:], ton[:], AF.Tanh)
                csb = cp.tile([128, n], F32, tag=f"c{lo}", name=f"c{lo}")
                # contrib = t_on*w - t_off*w
                nc.vector.scalar_tensor_tensor(
                    out=csb[:], in0=ton[:], scalar=w4_sb[:],
                    in1=toffw4_sb[:, lo:hi],
                    op0=OP.mult, op1=OP.subtract)
                nc.scalar.dma_start(out=contrib_d[:, lo:hi], in_=csb[:])

            ep_next = 0
            for g in range(N_GRP):
                if g >= WC_BUFS:
                    wc_dma(g, wts[g])
                wt = wts[g]
                for qi in range(GROUP_SIZES[g] // COLS):
                    j = offs[g] // COLS + qi       # global quad index
                    ps = pp.tile([128, 384], F32, tag="ps")
                    for c in range(DC):
                        for q in range(COLS):
                            ol = qi * COLS + q     # local o within group
                            nc.tensor.matmul(
                                ps[q * R_MAX:(q + 1) * R_MAX, :],
                                lhsT=tokT_sb[:, c * R_MAX:(c + 1) * R_MAX],
                                rhs=wt[:, ol * FR + c * 384:
                                       ol * FR + (c + 1) * 384],
                                start=(c == 0), stop=(c == DC - 1),
                                tile_position=(0, q * R_MAX),
                            )
                    z = zp.tile([128, 384], F32, tag="z")
                    ei = next(i for i, (lo, hi) in enumerate(EP) if j < hi)
                    lo = EP[ei][0]
                    nc.vector.scalar_tensor_tensor(
                        out=z[:], in0=ps[:], scalar=1.0, in1=dep4_sb[:],
                        op0=OP.mult, op1=OP.mult,
                        accum_out=u_t[ei][:, j - lo:j - lo + 1],
                    )
                    if ep_next < len(EP) and j == EP[ep_next][1] - 1:
                        epilogue(ep_next)
                        ep_next += 1

    nc.compile()
    _nc_cache["nc"] = nc
    return nc


def _shard_wc(Wc):
    """Per-core Wc layout: [OC, 128(p), FR] fp8e4, scaled by 8.
    free index = c*384 + e with d = c*128 + p (o-major so each group DMA
    reads one contiguous DRAM block)."""
    shards = []
    for k in range(NCORES):
        wck = (Wc[k * OC:(k + 1) * OC] * SCALE).astype(ml_dtypes.float8_e4m3)
        wck = wck.reshape(OC, DC, 128, 384).transpose(0, 2, 1, 3)
        shards.append(np.ascontiguousarray(wck).reshape(OC, 128, FR))
    return shards


def run_device(in_maps, trace=False, tmpdir=None):
    nc = _build_nc()
    return run_bass_kernel_spmd(nc, in_maps, list(range(NCORES)),
                                trace=trace, tmpdir=tmpdir)


def _make_in_maps(tok_sel, w_sel, wc_shards, bc):
    """tok_sel [R_MAX, D] f32, w_sel [R_MAX] f32."""
    # tokT[p, c*R_MAX + r] = tok_sel[r, c*128 + p]
    tokT = np.ascontiguousarray(
        tok_sel.T.reshape(DC, 128, R_MAX).transpose(1, 0, 2)
    ).reshape(128, DC * R_MAX).astype(ml_dtypes.bfloat16)
    dep = (np.tanh(tok_sel) / SCALE).astype(ml_dtypes.bfloat16)
    dep4 = np.concatenate([dep] * COLS, axis=0)            # [128, D]
    w4 = np.concatenate([w_sel] * COLS).reshape(128, 1).astype(np.float32)
    maps = []
    for k in range(NCORES):
        bck = bc[k * OC:(k + 1) * OC]
        bcr4 = np.concatenate([
            np.broadcast_to(bck[q::COLS], (R_MAX, NQ)) for q in range(COLS)
        ]).astype(np.float32)
        maps.append({
            "wc": wc_shards[k],
            "tokT": tokT,
            "dep4": dep4,
            "w4": w4,
            "bcr4": np.ascontiguousarray(bcr4),
        })
    return maps


def kernel(**inputs):
    tokens = np.asarray(inputs["tokens"])
    heads = np.asarray(inputs["dep_heads"])
    tok_table = np.asarray(inputs["tok_table"], dtype=np.float32)
    Wc = np.asarray(inputs["Wc"], dtype=np.float32)
    bc = np.asarray(inputs["bc"], dtype=np.float32)
    Wr = np.asarray(inputs["Wr"], dtype=np.float32)
    br = np.asarray(inputs["br"], dtype=np.float32)
    assert tokens.shape == (B, S) and Wc.shape == (D, D, D)

    # host index selection: rows that can reach an unmasked (head==0) output row
    zs = [np.nonzero(heads[b] == 0)[0] for b in range(B)]
    sel = [(b, int(s2), int(heads[b, s2]))
           for b in range(B)
           for s2 in np.nonzero(np.isin(heads[b], zs[b]))[0]]
    R = len(sel)

    wc_shards = _shard_wc(Wc)
    w_full = Wr[0]

    contribs = []
    toff = None
    for lo in range(0, max(R, 1), R_MAX):
        chunk = sel[lo:lo + R_MAX]
        tok_sel = np.zeros((R_MAX, D), dtype=np.float32)
        w_sel = np.zeros(R_MAX, dtype=np.float32)
        for i, (b, s2, _dest) in enumerate(chunk):
            tok_sel[i] = tok_table[tokens[b, s2]]
            w_sel[i] = w_full[s2]
        res = run_device(_make_in_maps(tok_sel, w_sel, wc_shards, bc)).results
        # contrib[p, j]: row r=p%R_MAX, local channel o=COLS*j+(p//R_MAX)
        ck = []
        for k in range(NCORES):
            c4 = res[k]["contrib"]
            c = np.empty((R_MAX, OC), dtype=np.float32)
            for q in range(COLS):
                c[:, q::COLS] = c4[q * R_MAX:(q + 1) * R_MAX]
            ck.append(c)
        contribs.append(np.concatenate(ck, axis=1))        # [R_MAX, D]
        tk = []
        for k in range(NCORES):
            t4 = res[k]["toff"]                            # [COLS, NQ]
            t = np.empty(OC, dtype=np.float32)
            for q in range(COLS):
                t[q::COLS] = t4[q]
            tk.append(t)
        toff = np.concatenate(tk)                          # [D]

    base = (toff * w_full.sum() + br[0]).astype(np.float32)
    out = np.zeros((B, S, D), dtype=np.float32)
    for b in range(B):
        out[b, zs[b]] = base
    for i, (b, _s2, dest) in enumerate(sel):
        out[b, dest] += contribs[i // R_MAX][i % R_MAX]
    return out


# revision 9
# speedup vs baseline: 2.4086x; 1.0113x over previous
"""Trainium2 Bass kernel for nn_Composer (gnn_message_passing).

Math (exact reformulation of the reference):
  out[b,s1,:] = (heads[b,s1]==0) * ( base + sum_{s2: heads[b,s2]==s1} w[s2]*(t_on[b,s2]-t_off) )
  t_on[b,s2]  = tanh(u[b,s2] + bc),  u[b,s2,o] = tok[b,s2] @ Wc[o] @ tanh(tok[b,s2])
  t_off       = tanh(bc),  base = t_off*sum(w) + br

Only rows s2 whose head lands on a row with head==0 contribute to the output,
so u is needed for a handful of rows (R ~ 4-16 of 4096). The unavoidable cost
is streaming the bilinear weight Wc once; it is quantized to fp8e4 on the host
(226 MB f32 -> 56.6 MB fp8; the bilinear term is a small correction on top of
the exactly-computed base, so e4m3 error lands ~1e-3 of the output scale, far
under the 2e-2 gate). Wc is scaled by 8 before quantization to keep values out
of the fp8 subnormal range; the 1/8 is folded into dep on the host.

Sharding: Wc split over the output dim O=384 across 8 cores (48 each, 7.08 MB
fp8/core). Each core computes its o-slice of u with 4-way column-tiled
matmuls: o-channels 4j..4j+3 run concurrently in PE array column groups
0/1/2/3 (PSUM partition quarters), each streaming its fp8 Wc slice as the
moving operand against the same bf16 tokT stationary chunk. This keeps the
per-group tensor time under the DMA pace even when the HAM clock gate holds
the PE at 1.2 GHz (per-group DMA waits re-throttle it). A fused DVE
multiply+reduce against dep (stacked 4x across partitions) produces u for all
four channels at once. The host does index selection, quantization, sharding,
and the final scatter.
"""
import numpy as np
import ml_dtypes

import concourse.bass as bass
import concourse.bacc as bacc
import concourse.mybir as mybir
from concourse.tile import TileContext
from concourse.tile_rust import add_dep_helper
from concourse.bass_utils import run_bass_kernel_spmd

F32 = mybir.dt.float32
BF16 = mybir.dt.bfloat16
FP8 = mybir.dt.float8e4

B, S, D = 8, 512, 384
NCORES = 8
OC = D // NCORES          # output channels per core = 48
COLS = 4                  # column-tiled concurrent o-channels per wave
NQ = OC // COLS           # o-channel quads per core = 12
DC = D // 128             # contraction chunks = 3
FR = DC * 384             # fp8 free-dim elements per o-channel = 1152
R_MAX = 128 // COLS       # padded selected-row capacity per device run = 32
SCALE = 8.0               # host folds Wc*8 / dep/8 to avoid fp8 subnormals
# Wc transfer group sizes in o-channels (multiples of COLS: channels are
# consumed in column-tiled quads). Small head group so compute starts early,
# big middle groups for DMA efficiency, small tail so the last DMA->compute
# chain is short.
GROUP_SIZES = [4, 4, 8, 8, 8, 8, 4, 4]
assert sum(GROUP_SIZES) == OC and all(g % COLS == 0 for g in GROUP_SIZES)
N_GRP = len(GROUP_SIZES)
WC_BUFS = N_GRP           # whole fp8 shard stays resident in SBUF

_nc_cache = {}


def _build_nc():
    if "nc" in _nc_cache:
        return _nc_cache["nc"]
    nc = bacc.Bacc("TRN2", target_bir_lowering=False, debug=False)
    # one DRAM tensor per transfer group, p-major [128, no*FR] so each group
    # DMA is a single fully-contiguous DRAM block with multi-KB runs per
    # partition on both sides
    wc_d = [nc.dram_tensor(f"wc{g}", [128, GROUP_SIZES[g] * FR], FP8,
                           kind="ExternalInput") for g in range(N_GRP)]
    tokT_d = nc.dram_tensor("tokT", [128, DC * R_MAX], BF16, kind="ExternalInput")
    dep4_d = nc.dram_tensor("dep4", [128, D], BF16, kind="ExternalInput")
    w4_d = nc.dram_tensor("w4", [128, 1], F32, kind="ExternalInput")
    bcr4_d = nc.dram_tensor("bcr4", [128, NQ], F32, kind="ExternalInput")
    contrib_d = nc.dram_tensor("contrib", [128, NQ], F32, kind="ExternalOutput")
    toff_d = nc.dram_tensor("toff", [COLS, NQ], F32, kind="ExternalOutput")

    AF = mybir.ActivationFunctionType
    OP = mybir.AluOpType

    # epilogue chunk boundaries in quad index space
    EP = [(0, 6), (6, 11), (11, 12)]

    with TileContext(nc) as tc:
        with (
            tc.tile_pool(name="const", bufs=1) as cp,
            tc.tile_pool(name="wcp", bufs=WC_BUFS) as wcp,
            tc.tile_pool(name="zp", bufs=16) as zp,
            tc.tile_pool(name="pp", bufs=4, space="PSUM") as pp,
        ):
            offs = [sum(GROUP_SIZES[:g]) for g in range(N_GRP)]

            # Wc stream owns the SP HWDGE ring; everything small goes through
            # the scalar engine's ring so it never queues behind megabytes.
            # All groups stay resident in SBUF (55 KB/partition), so every
            # group DMA is issued upfront and the ring never stalls on
            # buffer reuse.
            wts = []
            for g in range(N_GRP):
                wts.append(wcp.tile([128, GROUP_SIZES[g] * FR], FP8,
                                    tag="wc", name=f"wt{g}"))
            for g in range(N_GRP):
                nc.sync.dma_start(out=wts[g][:], in_=wc_d[g][:])

            tokT_sb = cp.tile([128, DC * R_MAX], BF16)
            nc.scalar.dma_start(out=tokT_sb[:], in_=tokT_d[:])
            dep4_sb = cp.tile([128, D], BF16)
            nc.scalar.dma_start(out=dep4_sb[:], in_=dep4_d[:])
            w4_sb = cp.tile([128, 1], F32)
            nc.scalar.dma_start(out=w4_sb[:], in_=w4_d[:])
            bcr4_sb = cp.tile([128, NQ], F32)
            nc.scalar.dma_start(out=bcr4_sb[:], in_=bcr4_d[:])

            toff4_sb = cp.tile([128, NQ], F32)
            nc.scalar.activation(toff4_sb[:], bcr4_sb[:], AF.Tanh)
            nc.scalar.dma_start(out=toff_d[:], in_=toff4_sb[0:128:R_MAX, :])
            # DVE observes dep4/w4 ticks here so the hot-loop reduce ops
            # carry few sync waits (each extra wait costs an event semaphore)
            dep_touch = cp.tile([128, 1], F32)
            nc.vector.tensor_copy(out=dep_touch[:], in_=dep4_sb[:, 0:1])
            # toffw4[p,j] = tanh(bc)[quad j, col p//32] * w[p%32]
            toffw4_sb = cp.tile([128, NQ], F32)
            nc.vector.tensor_scalar_mul(toffw4_sb[:], toff4_sb[:], w4_sb[:])

            u_t = [cp.tile([128, hi - lo], F32, tag=f"u{lo}", name=f"u{lo}")
                   for lo, hi in EP]

            def epilogue(ei):
                """contrib[:, lo:hi] = w*(tanh(u+bc) - t_off). For a single
                quad the +bc folds into the ACT bias port."""
                lo, hi = EP[ei]
                n = hi - lo
                ton = cp.tile([128, n], F32, tag=f"ton{lo}", name=f"ton{lo}")
                if n == 1:
                    nc.scalar.activation(ton[:], u_t[ei][:], AF.Tanh,
                                         bias=bcr4_sb[:, lo:lo + 1])
                else:
                    nc.vector.tensor_tensor(ton[:], u_t[ei][:],
                                            bcr4_sb[:, lo:hi], OP.add)
                    nc.scalar.activation(ton[:], ton[:], AF.Tanh)
                csb = cp.tile([128, n], F32, tag=f"c{lo}", name=f"c{lo}")
                # contrib = t_on*w - t_off*w
                nc.vector.scalar_tensor_tensor(
                    out=csb[:], in0=ton[:], scalar=w4_sb[:],
                    in1=toffw4_sb[:, lo:hi],
                    op0=OP.mult, op1=OP.subtract)
                nc.scalar.dma_start(out=contrib_d[:, lo:hi], in_=csb[:])

            ep_next = 0
            for g in range(N_GRP):
                wt = wts[g]
                for qi in range(GROUP_SIZES[g] // COLS):
                    j = offs[g] // COLS + qi       # global quad index
                    ps = pp.tile([128, 384], F32, tag="ps")
                    for c in range(DC):
                        for q in range(COLS):
                            ol = qi * COLS + q     # local o within group
                            nc.tensor.matmul(
                                ps[q * R_MAX:(q + 1) * R_MAX, :],
                                lhsT=tokT_sb[:, c * R_MAX:(c + 1) * R_MAX],
                                rhs=wt[:, ol * FR + c * 384:
                                       ol * FR + (c + 1) * 384],
                                start=(c == 0), stop=(c == DC - 1),
                                tile_position=(0, q * R_MAX),
                            )
                    z = zp.tile([128, 384], F32, tag="z")
                    ei = next(i for i, (lo, hi) in enumerate(EP) if j < hi)
                    lo = EP[ei][0]
                    nc.vector.scalar_tensor_tensor(
                        out=z[:], in0=ps[:], scalar=1.0, in1=dep4_sb[:],
                        op0=OP.mult, op1=OP.mult,
                        accum_out=u_t[ei][:, j - lo:j - lo + 1],
                    )
                    if ep_next < len(EP) and j == EP[ep_next][1] - 1:
                        epilogue(ep_next)
                        ep_next += 1

    nc.compile()
    _nc_cache["nc"] = nc
    return nc


def _shard_wc(Wc):
    """Per-core Wc as one array per transfer group: [128(p), no*FR] fp8e4,
    scaled by 8. Per-partition free layout [o][c][e] with d = c*128 + p, so
    each group is a single contiguous DRAM block."""
    shards = []
    for k in range(NCORES):
        wck = (Wc[k * OC:(k + 1) * OC] * SCALE).astype(ml_dtypes.float8_e4m3)
        wck = wck.reshape(OC, DC, 128, 384).transpose(2, 0, 1, 3)  # [p,o,c,e]
        groups = {}
        for g in range(N_GRP):
            o0 = sum(GROUP_SIZES[:g])
            blk = wck[:, o0:o0 + GROUP_SIZES[g]]
            groups[f"wc{g}"] = np.ascontiguousarray(blk).reshape(
                128, GROUP_SIZES[g] * FR)
        shards.append(groups)
    return shards


def run_device(in_maps, trace=False, tmpdir=None):
    nc = _build_nc()
    return run_bass_kernel_spmd(nc, in_maps, list(range(NCORES)),
                                trace=trace, tmpdir=tmpdir)


def _make_in_maps(tok_sel, w_sel, wc_shards, bc):
    """tok_sel [R_MAX, D] f32, w_sel [R_MAX] f32."""
    # tokT[p, c*R_MAX + r] = tok_sel[r, c*128 + p]
    tokT = np.ascontiguousarray(
        tok_sel.T.reshape(DC, 128, R_MAX).transpose(1, 0, 2)
    ).reshape(128, DC * R_MAX).astype(ml_dtypes.bfloat16)
    dep = (np.tanh(tok_sel) / SCALE).astype(ml_dtypes.bfloat16)
    dep4 = np.concatenate([dep] * COLS, axis=0)            # [128, D]
    w4 = np.concatenate([w_sel] * COLS).reshape(128, 1).astype(np.float32)
    maps = []
    for k in range(NCORES):
        bck = bc[k * OC:(k + 1) * OC]
        bcr4 = np.concatenate([
            np.broadcast_to(bck[q::COLS], (R_MAX, NQ)) for q in range(COLS)
        ]).astype(np.float32)
        maps.append({
            **wc_shards[k],
            "tokT": tokT,
            "dep4": dep4,
            "w4": w4,
            "bcr4": np.ascontiguousarray(bcr4),
        })
    return maps


def kernel(**inputs):
    tokens = np.asarray(inputs["tokens"])
    heads = np.asarray(inputs["dep_heads"])
    tok_table = np.asarray(inputs["tok_table"], dtype=np.float32)
    Wc = np.asarray(inputs["Wc"], dtype=np.float32)
    bc = np.asarray(inputs["bc"], dtype=np.float32)
    Wr = np.asarray(inputs["Wr"], dtype=np.float32)
    br = np.asarray(inputs["br"], dtype=np.float32)
    assert tokens.shape == (B, S) and Wc.shape == (D, D, D)

    # host index selection: rows that can reach an unmasked (head==0) output row
    zs = [np.nonzero(heads[b] == 0)[0] for b in range(B)]
    sel = [(b, int(s2), int(heads[b, s2]))
           for b in range(B)
           for s2 in np.nonzero(np.isin(heads[b], zs[b]))[0]]
    R = len(sel)

    wc_shards = _shard_wc(Wc)
    w_full = Wr[0]

    contribs = []
    toff = None
    for lo in range(0, max(R, 1), R_MAX):
        chunk = sel[lo:lo + R_MAX]
        tok_sel = np.zeros((R_MAX, D), dtype=np.float32)
        w_sel = np.zeros(R_MAX, dtype=np.float32)
        for i, (b, s2, _dest) in enumerate(chunk):
            tok_sel[i] = tok_table[tokens[b, s2]]
            w_sel[i] = w_full[s2]
        res = run_device(_make_in_maps(tok_sel, w_sel, wc_shards, bc)).results
        # contrib[p, j]: row r=p%R_MAX, local channel o=COLS*j+(p//R_MAX)
        ck = []
        for k in range(NCORES):
            c4 = res[k]["contrib"]
            c = np.empty((R_MAX, OC), dtype=np.float32)
            for q in range(COLS):
                c[:, q::COLS] = c4[q * R_MAX:(q + 1) * R_MAX]
            ck.append(c)
        contribs.append(np.concatenate(ck, axis=1))        # [R_MAX, D]
        tk = []
        for k in range(NCORES):
            t4 = res[k]["toff"]                            # [COLS, NQ]
            t = np.empty(OC, dtype=np.float32)
            for q in range(COLS):
                t[q::COLS] = t4[q]
            tk.append(t)
        toff = np.concatenate(tk)                          # [D]

    base = (toff * w_full.sum() + br[0]).astype(np.float32)
    out = np.zeros((B, S, D), dtype=np.float32)
    for b in range(B):
        out[b, zs[b]] = base
    for i, (b, _s2, dest) in enumerate(sel):
        out[b, dest] += contribs[i // R_MAX][i % R_MAX]
    return out
